# revision 5
# baseline (speedup 1.0000x reference)
"""Trainium2 Bass kernel for nn_KpcaStd (RBF-kernel PCA loss).

Computes, for x=input_data [8192,256], H [8192,512], D=inv_lambda_diag [512]:
    K = exp(-||x_i - x_j||^2 / 2)            [8192, 8192]
    E = H^T K                                 [512, 8192]
    s = -1/2 sum(D[:,None] * E^2) + 1/2 sum(E * H^T)
    out = s + 0.05 * s^2

Sharding: data-parallel over columns of K. Each of the 8 cores owns a
1024-column block K[:, c*1024:(c+1)*1024] (= rows c*1024.. of x), computes
the block, the partial E = H^T K_block [512, 1024], and partial weighted
sums, written out as per-partition partials [128, 2]. The host sums the
64 partials across cores/partitions and applies the final scalar map.

Device math per core (two 512-column passes to fit E in PSUM):
  PSUM t[i,j] = sq_j - 2*G[i,j] via PE:
     2 bf16 matmuls (x^T chunks) + 1 rank-2 matmul ([1;1] x [sqhi;sqlo])
     where sqhi/sqlo is a bf16 hi/lo split of sq (fp32 accuracy).
  K_tile = Exp(-0.5*t + (-0.5*sq_i)) on ScalarE, fp32 per-partition bias.
     sq is computed on host FROM THE bf16 x values, so the diagonal
     d2_ii = sq_i + sq_i - 2*sum(bf16(x)^2) cancels to ~0 exactly.
  E[h-block] += H_chunk^T @ K_tile accumulated in PSUM over 64 i-chunks.
  DVE: acc1 += D .* E^2 ; acc2 += E .* H^T ; final free-dim reduce.
"""

import os
import sys

import numpy as np

sys.path.insert(0, "/opt/trn_rl_repo")

import ml_dtypes

import concourse.bacc as bacc
import concourse.mybir as mybir
import concourse.tile as tile
from concourse.bass_utils import run_bass_kernel_spmd

BF16 = mybir.dt.bfloat16
F32 = mybir.dt.float32
NPBF16 = ml_dtypes.bfloat16

N = 8192  # rows of K / x
D = 256  # feature dim
HD = 512  # columns of H
NCORES = 8
JS = N // NCORES  # 1024 columns of K per core
NPASS = 2
JP = JS // NPASS  # 512 columns per pass (one PSUM bank)
NI = N // 128  # 64 i-chunks
NH = HD // 128  # 4 h-blocks

_cache = {}


def _build():
    """Build + schedule the single-core program (same on all 8 cores)."""
    nc = bacc.Bacc("TRN2", target_bir_lowering=False, debug=False)

    xtw_d = nc.dram_tensor("xtw", [NI, D, 128], BF16, kind="ExternalInput")
    xtr_d = nc.dram_tensor("xtr", [D, JS], BF16, kind="ExternalInput")
    h_d = nc.dram_tensor("hmat", [N, HD], BF16, kind="ExternalInput")
    ht_d = nc.dram_tensor("htl", [HD, JS], F32, kind="ExternalInput")
    aux_d = nc.dram_tensor("aux", [2, JS], BF16, kind="ExternalInput")
    nb_d = nc.dram_tensor("nbias", [128, NI], F32, kind="ExternalInput")
    dv_d = nc.dram_tensor("dvec", [128, NH], F32, kind="ExternalInput")
    out_d = nc.dram_tensor("partials", [128, 2], F32, kind="ExternalOutput")

    with tile.TileContext(nc) as tc:
        with (
            tc.tile_pool(name="xw", bufs=2 * NI) as xw_pool,
            tc.tile_pool(name="hp", bufs=NI) as h_pool,
            tc.tile_pool(name="cst", bufs=1) as cst_pool,
            tc.tile_pool(name="kt", bufs=3) as k_pool,
            tc.tile_pool(name="tmp", bufs=4) as tmp_pool,
            tc.tile_pool(name="gp", bufs=3, space="PSUM") as g_pool,
            tc.tile_pool(name="ep", bufs=NH, space="PSUM") as e_pool,
        ):
            xtr = [cst_pool.tile([128, JS], BF16, name=f"xtr{dc}", tag=f"xtr{dc}") for dc in range(2)]
            for dc in range(2):
                nc.sync.dma_start(
                    xtr[dc][:],
                    xtr_d.ap()[dc * 128 : (dc + 1) * 128, :],
                )
            aux = cst_pool.tile([2, JS], BF16)
            nc.sync.dma_start(aux[:], aux_d.ap()[:])
            nbias = cst_pool.tile([128, NI], F32)
            nc.sync.dma_start(nbias[:], nb_d.ap()[:])
            dvec = cst_pool.tile([128, NH], F32)
            nc.sync.dma_start(dvec[:], dv_d.ap()[:])
            ht = cst_pool.tile([128, NH * JS], F32)
            for hc in range(NH):
                nc.sync.dma_start(
                    ht[:, hc * JS : (hc + 1) * JS],
                    ht_d.ap()[hc * 128 : (hc + 1) * 128, :],
                )
            ones2 = cst_pool.tile([2, 128], BF16)
            nc.vector.memset(ones2[:], 1.0)

            # interleave weight/H chunk loads so compute can start early
            xw = []
            hts = []
            for ic in range(NI):
                w0 = xw_pool.tile([128, 128], BF16, name=f"xw0_{ic}", tag="xw")
                nc.sync.dma_start(w0[:], xtw_d.ap()[ic, 0:128, :])
                w1 = xw_pool.tile([128, 128], BF16, name=f"xw1_{ic}", tag="xw")
                nc.sync.dma_start(w1[:], xtw_d.ap()[ic, 128:256, :])
                xw.append((w0, w1))
                hh = h_pool.tile([128, HD], BF16, name=f"hch_{ic}", tag="hp")
                nc.sync.dma_start(
                    hh[:], h_d.ap()[ic * 128 : (ic + 1) * 128, :]
                )
                hts.append(hh)

            acc1 = cst_pool.tile([128, JP], F32)
            acc2 = cst_pool.tile([128, JP], F32)
            nc.vector.memset(acc1[:], 0.0)
            nc.vector.memset(acc2[:], 0.0)

            for jp in range(NPASS):
                j0 = jp * JP
                ep = [e_pool.tile([128, JP], F32, name=f"ep{jp}_{i}", tag="ep") for i in range(NH)]
                for ic in range(NI):
                    g = g_pool.tile([128, JP], F32, name=f"g_{jp}_{ic}", tag="gp")
                    nc.tensor.matmul(
                        g[:], xw[ic][0][:], xtr[0][:, j0 : j0 + JP],
                        start=True, stop=False,
                    )
                    nc.tensor.matmul(
                        g[:], xw[ic][1][:], xtr[1][:, j0 : j0 + JP],
                        start=False, stop=False,
                    )
                    nc.tensor.matmul(
                        g[:], ones2[:], aux[:, j0 : j0 + JP],
                        start=False, stop=True,
                    )
                    kt = k_pool.tile([128, JP], BF16, tag="kt")
                    nc.scalar.activation(
                        kt[:], g[:],
                        mybir.ActivationFunctionType.Exp,
                        bias=nbias[:, ic : ic + 1],
                        scale=-0.5,
                    )
                    for hc in range(NH):
                        nc.tensor.matmul(
                            ep[hc][:],
                            hts[ic][:, hc * 128 : (hc + 1) * 128],
                            kt[:],
                            start=(ic == 0),
                            stop=(ic == NI - 1),
                        )
                for hc in range(NH):
                    t1 = tmp_pool.tile([128, JP], F32, name=f"t1_{jp}_{hc}", tag="tmp")
                    nc.scalar.activation(
                        t1[:], ep[hc][:],
                        mybir.ActivationFunctionType.Square,
                    )
                    nc.vector.tensor_scalar_mul(
                        t1[:], t1[:], dvec[:, hc : hc + 1]
                    )
                    nc.vector.tensor_add(acc1[:], acc1[:], t1[:])
                    t2 = tmp_pool.tile([128, JP], F32, name=f"t2_{jp}_{hc}", tag="tmp")
                    nc.vector.tensor_mul(
                        t2[:], ep[hc][:], ht[:, hc * JS + j0 : hc * JS + j0 + JP]
                    )
                    nc.vector.tensor_add(acc2[:], acc2[:], t2[:])

            red = cst_pool.tile([128, 2], F32)
            nc.vector.reduce_sum(red[:, 0:1], acc1[:], axis=mybir.AxisListType.X)
            nc.vector.reduce_sum(red[:, 1:2], acc2[:], axis=mybir.AxisListType.X)
            nc.sync.dma_start(out_d.ap()[:], red[:])

    nc.compile()
    return nc


def _prep_inputs(input_data, H, inv_lambda_diag):
    x32 = np.asarray(input_data, dtype=np.float32)
    xb = x32.astype(NPBF16)
    xbf = xb.astype(np.float32)
    # row norms of the *bf16* x, in fp64->fp32 (matches PE G_ii closely)
    sq = (xbf.astype(np.float64) ** 2).sum(axis=1).astype(np.float32)
    sqhi = sq.astype(NPBF16)
    sqlo = (sq - sqhi.astype(np.float32)).astype(NPBF16)

    # weights: [64, 256, 128] — xtw[ic, d, p] = bf16(x)[ic*128+p, d]
    xtw = np.ascontiguousarray(
        xbf.reshape(NI, 128, D).transpose(0, 2, 1)
    ).astype(NPBF16)
    h16 = np.asarray(H, dtype=np.float32).astype(NPBF16)
    nbias = np.ascontiguousarray((-0.5 * sq).reshape(NI, 128).T).astype(
        np.float32
    )
    dvec = np.ascontiguousarray(
        np.asarray(inv_lambda_diag, dtype=np.float32).reshape(NH, 128).T
    ).astype(np.float32)

    in_maps = []
    for c in range(NCORES):
        sl = slice(c * JS, (c + 1) * JS)
        xtr = np.ascontiguousarray((-2.0 * xbf[sl]).T).astype(NPBF16)
        aux = np.ascontiguousarray(np.stack([sqhi[sl], sqlo[sl]]))
        htl = np.ascontiguousarray(
            np.asarray(H, dtype=np.float32)[sl].T
        ).astype(np.float32)
        in_maps.append(
            {
                "xtw": xtw,
                "xtr": xtr,
                "hmat": h16,
                "htl": htl,
                "aux": aux,
                "nbias": nbias,
                "dvec": dvec,
            }
        )
    return in_maps


def kernel(input_data, H, inv_lambda_diag, _want_profile=False):
    if "nc" not in _cache:
        _cache["nc"] = _build()
    nc = _cache["nc"]
    in_maps = _prep_inputs(input_data, H, inv_lambda_diag)

    trace = bool(_want_profile or os.environ.get("KPCA_TRACE"))
    res = run_bass_kernel_spmd(
        nc, in_maps, list(range(NCORES)), trace=trace,
        tmpdir=os.environ.get("KPCA_TRACE_DIR") or None,
    )
    _cache["last_result"] = res

    s1 = 0.0
    s2 = 0.0
    for c in range(NCORES):
        parts = res.results[c]["partials"].astype(np.float64)
        s1 += parts[:, 0].sum()
        s2 += parts[:, 1].sum()
    s = -0.5 * s1 + 0.5 * s2
    out = s + 0.05 * s * s
    return np.array(out, dtype=np.float32)
